# revision 35
# baseline (speedup 1.0000x reference)
"""Host-side precompute + Bass/Tile kernel for the quantum circuit
expectation problem (validated tile algebra: see check_v2.py).

Restructured circuit (per batch b), state S as 64x64 complex matrix
(rows i = qubits 0-5, cols j = qubits 6-11):
  init: S = 1/64 (uniform, real)
  for l in 0..3:
      S *= D1_b                  [diag, rank-1 phase table]
      S = K2_6 @ S @ K2_6^T
      S *= D2_b
      if l < 3:  S = U0_l @ S @ V0_l^T + U1_l @ S @ V1_l^T
  E = sum(S~ * (M~ S~))          [M = realified(O (x) I32)]

v2 device mapping (per core, 32 batches, 4 streams x 8 batches):
  A    [128,512]: p = r*64+i,  f = b*64+j
  oldB [128,512]: p = b2*64+j, f = c*128 + r*64+i   (b = 2c+b2)
  jmult-PSUM     : f = r*256 + c*64 + i  (r-major "pplanes")
- D1/D2 applied via PACKED tables: 2 DVE products + 2 GpSimd combines (D1)
  or products only with the combine FOLDED into the following left-multiply
  operators (D2 -> U0/U1/obs), precomputed host-side.
- All flips (128x128 transposes) done by the DMA xbar (dma_start_transpose),
  costing zero PE/ACT time.
- fp16 states/operators, fp32 PSUM accumulate.
"""
import numpy as np

N_QUBITS = 12
N_LAYERS = 4
BATCH = 256
N_CORES = 8
BPC = BATCH // N_CORES  # 32

_S2 = np.array([[1, 0], [0, 1j]], dtype=np.complex128)
_H2 = np.array([[1, 1], [1, -1]], dtype=np.complex128) / np.sqrt(2)
_I64 = np.eye(64)
_Z64 = np.zeros((64, 64))
_P1 = np.block([[_I64, -_I64], [_Z64, _Z64]])
_P2 = np.block([[_Z64, _Z64], [_I64, _I64]])


def _rx(t):
    c, s = np.cos(t / 2), -1j * np.sin(t / 2)
    return np.array([[c, s], [s, c]], dtype=np.complex128)


def _rz(t):
    e = np.exp(-0.5j * t)
    return np.array([[e, 0], [0, np.conj(e)]], dtype=np.complex128)


def _kron_list(ms):
    out = np.array([[1.0]], dtype=np.complex128)
    for m in ms:
        out = np.kron(out, m)
    return out


def _prefxor_perm(nbits):
    n = 2**nbits
    out = np.zeros(n, dtype=np.int64)
    for idx in range(n):
        acc, o = 0, 0
        for k in range(nbits):
            acc ^= (idx >> (nbits - 1 - k)) & 1
            o = (o << 1) | acc
        out[idx] = o
    return out


def _perm_matrix(perm):
    n = len(perm)
    P = np.zeros((n, n), dtype=np.complex128)
    P[perm, np.arange(n)] = 1.0
    return P


def build_operators(params):
    theta = np.asarray(params, np.float64).reshape(N_QUBITS, N_LAYERS, 3)
    K1_1q = _H2 @ _S2.conj().T
    K2_1q = _S2 @ _H2
    K1_6 = _kron_list([K1_1q] * 6)
    K2_6 = _kron_list([K2_1q] * 6)
    cA = _perm_matrix(_prefxor_perm(6))
    cB = _perm_matrix(_prefxor_perm(6))

    T = np.zeros((N_QUBITS, N_LAYERS, 2, 2), dtype=np.complex128)
    for q in range(N_QUBITS):
        for l in range(N_LAYERS):
            T[q, l] = _rx(theta[q, l, 2]) @ _rz(theta[q, l, 1]) @ _rx(theta[q, l, 0])

    K1e = K1_6.copy()
    K1o = K1_6.copy()
    lsb = np.arange(64) & 1
    K1e[:, lsb == 1] = 0.0
    K1o[:, lsb == 0] = 0.0
    Pi32 = _perm_matrix(np.arange(64) ^ 32)

    layers = []
    for l in range(3):
        TA = _kron_list([T[q, l] for q in range(6)])
        TB = _kron_list([T[q, l] for q in range(6, 12)])
        U0 = K1e @ cA @ TA
        U1 = K1o @ cA @ TA
        W0 = K1_6 @ cB
        V0 = W0 @ TB
        V1 = W0 @ Pi32 @ TB
        layers.append((U0, U1, V0, V1))

    Z = np.diag([1.0, -1.0]).astype(np.complex128)
    O = T[0, 3].conj().T @ Z @ T[0, 3]
    return dict(K2_6=K2_6, layers=layers, Omat=O)


def _realified(L):
    return np.block([[L.real, -L.imag], [L.imag, L.real]])


def _jside_Ts(L):
    Wr = np.kron(np.eye(2), L.real)
    Wi = np.kron(np.eye(2), L.imag)
    return (np.ascontiguousarray(Wr.T), np.ascontiguousarray(Wi.T),
            np.ascontiguousarray(-Wi.T))


PACK1_ORDER = ["ident", "k2l", "k2r_0", "k2r_1", "k2r_2"]
PACK2_ORDER = (["of_0", "of_1", "of_2", "of_3"]
               + [f"uf_{l}_{k}" for l in range(3) for k in (0, 2, 1, 3)]
               + [f"v{v}_{l}_{k}" for l in range(3) for v in (0, 1)
                  for k in range(3)])


def build_host_data(x, params):
    """Returns in_maps: list (per core) of dict name -> np.ndarray."""
    ops = build_operators(params)
    K2 = ops["K2_6"]

    pieces = {}
    pieces["ident"] = np.eye(128)
    pieces["k2l"] = np.ascontiguousarray(_realified(K2).T)
    k2r = _jside_Ts(K2)
    for k in range(3):
        pieces[f"k2r_{k}"] = k2r[k]
    M = _realified(np.kron(ops["Omat"], np.eye(32)))
    for k, tbl in enumerate([(_P1.T @ M @ _P1).T, (_P1.T @ M @ _P2).T,
                             (_P2.T @ M @ _P1).T, (_P2.T @ M @ _P2).T]):
        pieces[f"of_{k}"] = np.ascontiguousarray(tbl)
    for l in range(3):
        U0, U1, V0, V1 = ops["layers"][l]
        W0r, W1r = _realified(U0), _realified(U1)
        for k, tbl in enumerate([(W0r @ _P1).T, (W0r @ _P2).T,
                                 (W1r @ _P1).T, (W1r @ _P2).T]):
            pieces[f"uf_{l}_{k}"] = np.ascontiguousarray(tbl)
        for v, Vm in ((0, V0), (1, V1)):
            trip = _jside_Ts(Vm)
            for k in range(3):
                pieces[f"v{v}_{l}_{k}"] = trip[k]

    cpack = np.concatenate([pieces[nm] for nm in PACK1_ORDER],
                           axis=1).astype(np.float16)
    upack = np.concatenate([pieces[nm] for nm in PACK2_ORDER],
                           axis=1).astype(np.float16)

    pc6 = np.array([bin(i).count("1") for i in range(64)])
    pc12 = pc6[:, None] + pc6[None, :]
    x = np.asarray(x, dtype=np.float64)
    x1 = np.arcsin(x)
    x2 = np.arccos(x * x)

    in_maps = []
    for core in range(N_CORES):
        xs1 = x1[core * BPC:(core + 1) * BPC]
        xs2 = x2[core * BPC:(core + 1) * BPC]
        ph1 = np.exp(-1j * xs1[:, None, None] * (6 - pc12[None]))  # [32,i,j]
        ph2 = np.exp(-1j * xs2[:, None, None] * (6 - pc12[None]))

        # T1B/T2B [128, 2048]: p=(b2,j), f=(h, r, c4, i) — per-stream
        # contiguous [*, 512h:512h+512] blocks, (r,c,i)-major inside
        d1v = ph1.reshape(4, 4, 2, 64, 64)        # h,c4,b2,i,j
        T1B = np.empty((128, 2048))
        T2B = np.empty((128, 2048))
        t1v = T1B.reshape(2, 64, 4, 2, 4, 64)     # b2,j,h,r,c4,i
        t2v = T2B.reshape(2, 64, 4, 2, 4, 64)
        re = d1v.real.transpose(2, 4, 0, 1, 3)    # b2,j,h,c4,i
        im = d1v.imag.transpose(2, 4, 0, 1, 3)
        t1v[:, :, :, 0] = re
        t1v[:, :, :, 1] = im
        t2v[:, :, :, 0] = im
        t2v[:, :, :, 1] = re

        # T1A/T2A [128, 2048]: p=(r,i), f=(b32,j)
        phA = ph2.transpose(1, 0, 2).reshape(64, 2048)   # [i, (b,j)]
        T1A = np.concatenate([phA.real, phA.imag], axis=0)
        T2A = np.concatenate([phA.imag, phA.real], axis=0)

        m = {
            "cpack": cpack, "upack": upack,
            "t1b": T1B.astype(np.float16), "t2b": T2B.astype(np.float16),
            "t1a": T1A.astype(np.float16), "t2a": T2A.astype(np.float16),
        }
        in_maps.append(m)
    return in_maps


def pack_offsets(order):
    offs = {}
    off = 0
    for nm in order:
        offs[nm] = (off, 128)
        off += 128
    return offs, off


# ---------------------------------------------------------------- bass kernel
def emit(ctx, tc, dram):
    """Emit kernel IR. dram: dict name -> bass.AP (+ 'out' [1,32] f32)."""
    import concourse.mybir as mybir

    nc = tc.nc
    FP = mybir.dt.float32
    FW = mybir.dt.float16
    ALU = mybir.AluOpType

    consts = ctx.enter_context(tc.tile_pool(name="consts", bufs=1))
    states = ctx.enter_context(tc.tile_pool(name="states", bufs=4))
    temps = ctx.enter_context(tc.tile_pool(name="temps", bufs=4))
    psums = ctx.enter_context(tc.tile_pool(name="psums", bufs=4, space="PSUM"))
    pvs = ctx.enter_context(tc.tile_pool(name="pvs", bufs=2, space="PSUM"))
    tpsums = ctx.enter_context(tc.tile_pool(name="tpsums", bufs=1, space="PSUM"))

    offs1, packw1 = pack_offsets(PACK1_ORDER)
    offs2, packw2 = pack_offsets(PACK2_ORDER)
    CP = consts.tile([128, packw1], FW, name="CP", uniquify=False)
    UP = consts.tile([128, packw2], FW, name="UP", uniquify=False)

    def cslice(nm):
        if nm in offs1:
            o, w = offs1[nm]
            return CP[:, o:o + w]
        o, w = offs2[nm]
        return UP[:, o:o + w]

    # diag tables
    t1b = consts.tile([128, 2048], FW, name="c_t1b", uniquify=False)
    t2b = consts.tile([128, 2048], FW, name="c_t2b", uniquify=False)
    t1a = consts.tile([128, 2048], FW, name="c_t1a", uniquify=False)
    t2a = consts.tile([128, 2048], FW, name="c_t2a", uniquify=False)
    # load order: t1b (layer-0 shortcut) first, then packs, then the rest
    for q in range(4):
        qs = slice(512 * q, 512 * q + 512)
        nc.sync.dma_start(t1b[:, qs], dram["t1b"][:, qs])
    nc.sync.dma_start(CP[:, :], dram["cpack"][:, :])
    nc.sync.dma_start(UP[:, :], dram["upack"][:, :])
    nc.sync.dma_start(t1a[:, :], dram["t1a"][:, :])
    nc.sync.dma_start(t2a[:, :], dram["t2a"][:, :])
    nc.sync.dma_start(t2b[:, :], dram["t2b"][:, :])

    k2l = cslice("k2l")
    k2r = [cslice(f"k2r_{k}") for k in range(3)]
    of = [cslice(f"of_{k}") for k in range(4)]

    def cpair(nm):
        # 256-wide slice covering two adjacent pack entries
        o, _ = offs2[nm]
        return UP[:, o:o + 256]

    # ufp[l][0] = [uf0|uf2] (T1 tables, both branches), ufp[l][1] = [uf1|uf3]
    ufp = [[cpair(f"uf_{l}_0"), cpair(f"uf_{l}_1")] for l in range(3)]
    v0 = [[cslice(f"v0_{l}_{k}") for k in range(3)] for l in range(3)]
    v1 = [[cslice(f"v1_{l}_{k}") for k in range(3)] for l in range(3)]

    ones = consts.tile([128, 1], FW, name="ones", uniquify=False)
    nc.vector.memset(ones[:, :], 1.0)

    NS = 4
    FQ = 512

    def t1b_h(tbl, h):
        # per-stream contiguous [128,512] block, (r,c4,i)-major inside
        return tbl[:, 512 * h:512 * h + 512]

    def rci(t2d):
        return t2d[:, :].rearrange("p (r c i) -> p r c i", r=2, i=64)

    def cri(t2d):
        return t2d[:, :].rearrange("p (c r i) -> p r c i", r=2, i=64)

    def jmult(trip, src, dst_psum):
        # src (r,c,i)-major oldB state [128,512]; Wr serves both planes in
        # one paired N=512 matmul (out re|im adjacent, rhs re|im adjacent)
        WrT, WiT, WmiT = trip
        sre, sim = src[:, 0:256], src[:, 256:512]
        o_re = dst_psum[:, 0:256]
        o_im = dst_psum[:, 256:512]
        nc.tensor.matmul(dst_psum[:, 0:512], WrT, src[:, 0:512],
                         start=True, stop=False, skip_group_check=True)
        nc.tensor.matmul(o_re, WmiT, sim, start=False, stop=True,
                         skip_group_check=True)
        nc.tensor.matmul(o_im, WiT, sre, start=False, stop=True,
                         skip_group_check=True)

    def bplanes(t):
        # oldB (c,r,i)-major tile -> plane views [128, 4, 64] strided
        v = t[:, :].rearrange("p (c r i) -> p r c i", r=2, i=64)
        return v[:, 0], v[:, 1]

    def jmult2(trip0, src0, trip1, src1, dst_psum):
        # Wr of each branch serves both output planes via an r-major-strided
        # full-width rhs view; only the Wi cross terms stay per-plane
        s0re, s0im = bplanes(src0)
        s1re, s1im = bplanes(src1)
        s0rm = src0[:, :].rearrange("p (c r i) -> p r c i", r=2, i=64)
        s1rm = src1[:, :].rearrange("p (c r i) -> p r c i", r=2, i=64)
        o_re = dst_psum[:, 0:256]
        o_im = dst_psum[:, 256:512]
        full = dst_psum[:, 0:512]
        nc.tensor.matmul(full, trip0[0], s0rm, start=True, stop=False,
                         skip_group_check=True)
        nc.tensor.matmul(full, trip1[0], s1rm, start=False, stop=False,
                         skip_group_check=True)
        nc.tensor.matmul(o_re, trip0[2], s0im, start=False, stop=False,
                         skip_group_check=True)
        nc.tensor.matmul(o_re, trip1[2], s1im, start=False, stop=True,
                         skip_group_check=True)
        nc.tensor.matmul(o_im, trip0[1], s0re, start=False, stop=False,
                         skip_group_check=True)
        nc.tensor.matmul(o_im, trip1[1], s1re, start=False, stop=True,
                         skip_group_check=True)

    ident = cslice("ident")

    def flip(src, dst, copy_eng):
        # 4 back-to-back PE transposes (ident stays stationary), then one
        # PSUM->SBUF copy on the given engine
        tp = tpsums.tile([128, FQ], FW, name="tp", tag="tp")
        for c in range(4):
            nc.tensor.transpose(tp[:, 128 * c:128 * c + 128],
                                src[:, 128 * c:128 * c + 128], ident)
        copy_eng(dst[:, :], tp[:, :])

    # ---------------- pipeline ------------------------------------------
    pv_tiles = [None] * NS
    res = states.tile([1, 32], FP, name="res", tag="res", bufs=1)

    for l in range(4):
        SB2s = [None] * NS
        SB3s = [None] * NS
        SAs = [None] * NS
        T1s = [None] * NS
        T2s = [None] * NS
        for h in range(NS):
            SB2s[h] = states.tile([128, FQ], FW, name=f"SB2_{h}", tag="SB2")
            if l == 0:
                nc.scalar.mul(SB2s[h][:, :], t1b_h(t1b, h), 1.0 / 64.0)
            else:
                pv = pv_tiles[h]
                pvc = temps.tile([128, FQ], FW, name="pvc", tag="pvc")
                nc.vector.tensor_copy(pvc[:, :], pv[:, :])
                d1 = temps.tile([128, FQ], FW, name="d1", tag="d1")
                d2 = temps.tile([128, FQ], FW, name="d2", tag="d2")
                nc.vector.tensor_tensor(d1[:, :], pvc[:, :], t1b_h(t1b, h),
                                        ALU.mult)
                nc.gpsimd.tensor_tensor(d2[:, :], pvc[:, :], t1b_h(t2b, h),
                                        ALU.mult)
                nc.gpsimd.tensor_tensor(SB2s[h][:, 0:256], d1[:, 0:256],
                                        d1[:, 256:512], ALU.subtract)
                nc.gpsimd.tensor_tensor(SB2s[h][:, 256:512], d2[:, 0:256],
                                        d2[:, 256:512], ALU.add)
        for h in range(NS):
            pk = psums.tile([128, FQ], FP, name=f"pk{h}", tag="pstate")
            jmult(k2r, SB2s[h], pk)
            SB3s[h] = states.tile([128, FQ], FW, name=f"SB3_{h}", tag="SB3")
            nc.scalar.copy(cri(SB3s[h]), rci(pk))
        for h in range(NS):
            SAs[h] = states.tile([128, FQ], FW, name=f"SA{h}", tag="SA")
            flip(SB3s[h], SAs[h], nc.scalar.copy)
        for h in range(NS):
            pl = psums.tile([128, FQ], FP, name=f"pl{h}", tag="pstate")
            nc.tensor.matmul(pl[:, :], k2l, SAs[h][:, :], start=True, stop=True)
            hs = slice(FQ * h, FQ * h + FQ)
            plc = temps.tile([128, FQ], FW, name="plc", tag="plc")
            nc.vector.tensor_copy(plc[:, :], pl[:, :])
            T1s[h] = temps.tile([128, FQ], FW, name="t1s", tag="t1s")
            T2s[h] = temps.tile([128, FQ], FW, name="t2s", tag="t2s")
            nc.vector.tensor_tensor(T1s[h][:, :], plc[:, :], t1a[:, hs], ALU.mult)
            nc.vector.tensor_tensor(T2s[h][:, :], plc[:, :], t2a[:, hs], ALU.mult)

        if l == 3:
            for h in range(NS):
                po1 = psums.tile([128, FQ], FP, name=f"po1{h}", tag="pstate")
                nc.tensor.matmul(po1[:, :], of[0], T1s[h][:, :], start=True,
                                 stop=False)
                nc.tensor.matmul(po1[:, :], of[1], T2s[h][:, :], start=False,
                                 stop=True)
                po2 = pvs.tile([128, FQ], FP, name=f"po2{h}", tag="pv")
                nc.tensor.matmul(po2[:, :], of[2], T1s[h][:, :], start=True,
                                 stop=False)
                nc.tensor.matmul(po2[:, :], of[3], T2s[h][:, :], start=False,
                                 stop=True)
                PR1 = states.tile([128, FQ], FW, name=f"PR1{h}", tag="SB3")
                PR2 = states.tile([128, FQ], FW, name=f"PR2{h}", tag="SA")
                nc.vector.tensor_tensor(PR1[:, :], T1s[h][:, :], po1[:, :],
                                        ALU.mult)
                nc.vector.tensor_tensor(PR2[:, :], T2s[h][:, :], po2[:, :],
                                        ALU.mult)
                ep = tpsums.tile([1, FQ], FP, name="ep", tag="ep", bufs=1)
                nc.tensor.matmul(ep[:, :], ones[:, :], PR1[:, :], start=True,
                                 stop=False)
                nc.tensor.matmul(ep[:, :], ones[:, :], PR2[:, :], start=False,
                                 stop=True)
                epv = ep[:, :].rearrange("p (b j) -> p b j", j=64)
                nc.vector.tensor_reduce(res[:, 8 * h:8 * h + 8], epv,
                                        axis=mybir.AxisListType.X, op=ALU.add)
        else:
            SBas = [None] * NS
            SBbs = [None] * NS
            for h in range(NS):
                # fused U-side: stationary = D2-product chunk, rhs = paired
                # folded tables; out chunk = [flip(U0-res) | flip(U1-res)]
                pbx1 = psums.tile([128, FQ], FP, name=f"pbx1{h}", tag="pstate")
                pbx2 = psums.tile([128, FQ], FP, name=f"pbx2{h}", tag="pstate")
                for c in range(4):
                    cs = slice(128 * c, 128 * c + 128)
                    pbx = pbx1 if c < 2 else pbx2
                    os_ = slice(256 * (c % 2), 256 * (c % 2) + 256)
                    nc.tensor.matmul(pbx[:, os_], T1s[h][:, cs], ufp[l][0],
                                     start=True, stop=False)
                    nc.tensor.matmul(pbx[:, os_], T2s[h][:, cs], ufp[l][1],
                                     start=False, stop=True)
                SBas[h] = states.tile([128, FQ], FW, name=f"SBa{h}", tag="SB3")
                SBbs[h] = states.tile([128, FQ], FW, name=f"SBb{h}", tag="SA")
                v1_ = pbx1[:, :].rearrange("p (cc s i) -> p s cc i", s=2, i=128)
                v2_ = pbx2[:, :].rearrange("p (cc s i) -> p s cc i", s=2, i=128)
                a_ = SBas[h][:, :].rearrange("p (cc i) -> p cc i", i=128)
                b_ = SBbs[h][:, :].rearrange("p (cc i) -> p cc i", i=128)
                nc.scalar.copy(a_[:, 0:2], v1_[:, 0])
                nc.scalar.copy(a_[:, 2:4], v2_[:, 0])
                nc.vector.tensor_copy(b_[:, 0:2], v1_[:, 1])
                nc.vector.tensor_copy(b_[:, 2:4], v2_[:, 1])
            for h in range(NS):
                pv = pvs.tile([128, FQ], FP, name=f"pv{h}", tag="pv")
                jmult2(v0[l], SBas[h], v1[l], SBbs[h], pv)
                pv_tiles[h] = pv

    nc.sync.dma_start(dram["out"][:, :], res[:, :])


# ======================================================================
# public entry point
# ======================================================================
_CACHED = {}


def _build_program(use_f32r=True):
    """Build + compile the (input-independent) bass program once."""
    key = True
    if key in _CACHED:
        return _CACHED[key]
    from contextlib import ExitStack
    import concourse.bacc as bacc
    import concourse.mybir as mybir
    import concourse.tile as tile

    nc = bacc.Bacc("TRN2", target_bir_lowering=False, debug=False,
                   enable_asserts=True)
    _, packw1 = pack_offsets(PACK1_ORDER)
    _, packw2 = pack_offsets(PACK2_ORDER)
    shapes = {"cpack": [128, packw1], "upack": [128, packw2],
              "t1b": [128, 2048], "t2b": [128, 2048],
              "t1a": [128, 2048], "t2a": [128, 2048]}
    dram = {}
    for name, shape in shapes.items():
        dram[name] = nc.dram_tensor(
            name, shape, mybir.dt.float16, kind="ExternalInput").ap()
    dram["out"] = nc.dram_tensor("out", [1, 32], mybir.dt.float32,
                                 kind="ExternalOutput").ap()
    with tile.TileContext(nc) as tc:
        with ExitStack() as ctx:
            emit(ctx, tc, dram)
    nc.compile()
    _CACHED[key] = nc
    return nc


def kernel(x, params):
    """Full-input entry point: x (256,) f32, params (144,) f32 -> (256,) f32.

    Shards the batch over 8 NeuronCores (32 per core), runs the Bass/Tile
    statevector kernel SPMD, gathers per-core expectation values.
    """
    from concourse.bass_utils import run_bass_kernel_spmd

    x = np.asarray(x, dtype=np.float32).reshape(BATCH)
    params = np.asarray(params, dtype=np.float32).reshape(N_QUBITS * N_LAYERS * 3)
    nc = _build_program()
    in_maps = build_host_data(x, params)
    res = run_bass_kernel_spmd(nc, in_maps, list(range(N_CORES)))
    out = np.concatenate([res.results[c]["out"].reshape(BPC)
                          for c in range(N_CORES)])
    return out.astype(np.float32)


# revision 39
# speedup vs baseline: 1.2396x; 1.2396x over previous
"""Host-side precompute + Bass/Tile kernel for the quantum circuit
expectation problem (validated tile algebra: see check_v2.py).

Restructured circuit (per batch b), state S as 64x64 complex matrix
(rows i = qubits 0-5, cols j = qubits 6-11):
  init: S = 1/64 (uniform, real)
  for l in 0..3:
      S *= D1_b                  [diag, rank-1 phase table]
      S = K2_6 @ S @ K2_6^T
      S *= D2_b
      if l < 3:  S = U0_l @ S @ V0_l^T + U1_l @ S @ V1_l^T
  E = sum(S~ * (M~ S~))          [M = realified(O (x) I32)]

v2 device mapping (per core, 32 batches, 4 streams x 8 batches):
  A    [128,512]: p = r*64+i,  f = b*64+j
  oldB [128,512]: p = b2*64+j, f = c*128 + r*64+i   (b = 2c+b2)
  jmult-PSUM     : f = r*256 + c*64 + i  (r-major "pplanes")
- D1/D2 applied via PACKED tables: 2 DVE products + 2 GpSimd combines (D1)
  or products only with the combine FOLDED into the following left-multiply
  operators (D2 -> U0/U1/obs), precomputed host-side.
- All flips (128x128 transposes) done by the DMA xbar (dma_start_transpose),
  costing zero PE/ACT time.
- fp16 states/operators, fp32 PSUM accumulate.
"""
import numpy as np

N_QUBITS = 12
N_LAYERS = 4
BATCH = 256
N_CORES = 8
BPC = BATCH // N_CORES  # 32

_S2 = np.array([[1, 0], [0, 1j]], dtype=np.complex128)
_H2 = np.array([[1, 1], [1, -1]], dtype=np.complex128) / np.sqrt(2)
_I64 = np.eye(64)
_Z64 = np.zeros((64, 64))
_P1 = np.block([[_I64, -_I64], [_Z64, _Z64]])
_P2 = np.block([[_Z64, _Z64], [_I64, _I64]])


def _rx(t):
    c, s = np.cos(t / 2), -1j * np.sin(t / 2)
    return np.array([[c, s], [s, c]], dtype=np.complex128)


def _rz(t):
    e = np.exp(-0.5j * t)
    return np.array([[e, 0], [0, np.conj(e)]], dtype=np.complex128)


def _kron_list(ms):
    out = np.array([[1.0]], dtype=np.complex128)
    for m in ms:
        out = np.kron(out, m)
    return out


def _prefxor_perm(nbits):
    n = 2**nbits
    out = np.zeros(n, dtype=np.int64)
    for idx in range(n):
        acc, o = 0, 0
        for k in range(nbits):
            acc ^= (idx >> (nbits - 1 - k)) & 1
            o = (o << 1) | acc
        out[idx] = o
    return out


def _perm_matrix(perm):
    n = len(perm)
    P = np.zeros((n, n), dtype=np.complex128)
    P[perm, np.arange(n)] = 1.0
    return P


def build_operators(params):
    theta = np.asarray(params, np.float64).reshape(N_QUBITS, N_LAYERS, 3)
    K1_1q = _H2 @ _S2.conj().T
    K2_1q = _S2 @ _H2
    K1_6 = _kron_list([K1_1q] * 6)
    K2_6 = _kron_list([K2_1q] * 6)
    cA = _perm_matrix(_prefxor_perm(6))
    cB = _perm_matrix(_prefxor_perm(6))

    T = np.zeros((N_QUBITS, N_LAYERS, 2, 2), dtype=np.complex128)
    for q in range(N_QUBITS):
        for l in range(N_LAYERS):
            T[q, l] = _rx(theta[q, l, 2]) @ _rz(theta[q, l, 1]) @ _rx(theta[q, l, 0])

    K1e = K1_6.copy()
    K1o = K1_6.copy()
    lsb = np.arange(64) & 1
    K1e[:, lsb == 1] = 0.0
    K1o[:, lsb == 0] = 0.0
    Pi32 = _perm_matrix(np.arange(64) ^ 32)

    layers = []
    for l in range(3):
        TA = _kron_list([T[q, l] for q in range(6)])
        TB = _kron_list([T[q, l] for q in range(6, 12)])
        U0 = K1e @ cA @ TA
        U1 = K1o @ cA @ TA
        W0 = K1_6 @ cB
        V0 = W0 @ TB
        V1 = W0 @ Pi32 @ TB
        layers.append((U0, U1, V0, V1))

    Z = np.diag([1.0, -1.0]).astype(np.complex128)
    O = T[0, 3].conj().T @ Z @ T[0, 3]
    return dict(K2_6=K2_6, layers=layers, Omat=O)


def _realified(L):
    return np.block([[L.real, -L.imag], [L.imag, L.real]])


def _jside_Ts(L):
    Wr = np.kron(np.eye(2), L.real)
    Wi = np.kron(np.eye(2), L.imag)
    return (np.ascontiguousarray(Wr.T), np.ascontiguousarray(Wi.T),
            np.ascontiguousarray(-Wi.T))


PACK1_ORDER = ["ident", "k2l", "k2r_0", "k2r_1", "k2r_2"]
PACK2_ORDER = (["of_0", "of_1", "of_2", "of_3"]
               + [f"uf_{l}_{k}" for l in range(3) for k in (0, 2, 1, 3)]
               + [f"v{v}_{l}_{k}" for l in range(3) for v in (0, 1)
                  for k in range(3)])


def build_host_data(x, params):
    """Returns in_maps: list (per core) of dict name -> np.ndarray."""
    ops = build_operators(params)
    K2 = ops["K2_6"]

    pieces = {}
    pieces["ident"] = np.eye(128)
    pieces["k2l"] = np.ascontiguousarray(_realified(K2).T)
    k2r = _jside_Ts(K2)
    for k in range(3):
        pieces[f"k2r_{k}"] = k2r[k]
    M = _realified(np.kron(ops["Omat"], np.eye(32)))
    for k, tbl in enumerate([(_P1.T @ M @ _P1).T, (_P1.T @ M @ _P2).T,
                             (_P2.T @ M @ _P1).T, (_P2.T @ M @ _P2).T]):
        pieces[f"of_{k}"] = np.ascontiguousarray(tbl)
    for l in range(3):
        U0, U1, V0, V1 = ops["layers"][l]
        W0r, W1r = _realified(U0), _realified(U1)
        for k, tbl in enumerate([(W0r @ _P1).T, (W0r @ _P2).T,
                                 (W1r @ _P1).T, (W1r @ _P2).T]):
            pieces[f"uf_{l}_{k}"] = np.ascontiguousarray(tbl)
        for v, Vm in ((0, V0), (1, V1)):
            trip = _jside_Ts(Vm)
            for k in range(3):
                pieces[f"v{v}_{l}_{k}"] = trip[k]

    cpack = np.concatenate([pieces[nm] for nm in PACK1_ORDER],
                           axis=1).astype(np.float16)
    upack = np.concatenate([pieces[nm] for nm in PACK2_ORDER],
                           axis=1).astype(np.float16)

    pc6 = np.array([bin(i).count("1") for i in range(64)])
    pc12 = pc6[:, None] + pc6[None, :]
    x = np.asarray(x, dtype=np.float64)
    x1 = np.arcsin(x)
    x2 = np.arccos(x * x)

    in_maps = []
    for core in range(N_CORES):
        xs1 = x1[core * BPC:(core + 1) * BPC]
        xs2 = x2[core * BPC:(core + 1) * BPC]
        ph1 = np.exp(-1j * xs1[:, None, None] * (6 - pc12[None]))  # [32,i,j]
        ph2 = np.exp(-1j * xs2[:, None, None] * (6 - pc12[None]))

        # T1B/T2B [128, 2048]: p=(b2,j), f=(h, r, c4, i) — per-stream
        # contiguous [*, 512h:512h+512] blocks, (r,c,i)-major inside
        d1v = ph1.reshape(4, 4, 2, 64, 64)        # h,c4,b2,i,j
        T1B = np.empty((128, 2048))
        T2B = np.empty((128, 2048))
        t1v = T1B.reshape(2, 64, 4, 2, 4, 64)     # b2,j,h,r,c4,i
        t2v = T2B.reshape(2, 64, 4, 2, 4, 64)
        re = d1v.real.transpose(2, 4, 0, 1, 3)    # b2,j,h,c4,i
        im = d1v.imag.transpose(2, 4, 0, 1, 3)
        t1v[:, :, :, 0] = re
        t1v[:, :, :, 1] = im
        t2v[:, :, :, 0] = im
        t2v[:, :, :, 1] = re

        # T1A/T2A [128, 2048]: p=(r,i), f=(b32,j)
        phA = ph2.transpose(1, 0, 2).reshape(64, 2048)   # [i, (b,j)]
        T1A = np.concatenate([phA.real, phA.imag], axis=0)
        T2A = np.concatenate([phA.imag, phA.real], axis=0)

        m = {
            "cpack": cpack, "upack": upack,
            "t1b": T1B.astype(np.float16), "t2b": T2B.astype(np.float16),
            "t1a": T1A.astype(np.float16), "t2a": T2A.astype(np.float16),
        }
        in_maps.append(m)
    return in_maps


def pack_offsets(order):
    offs = {}
    off = 0
    for nm in order:
        offs[nm] = (off, 128)
        off += 128
    return offs, off


# ---------------------------------------------------------------- bass kernel
def emit(ctx, tc, dram):
    """Emit kernel IR. dram: dict name -> bass.AP (+ 'out' [1,32] f32)."""
    import concourse.mybir as mybir

    nc = tc.nc
    FP = mybir.dt.float32
    FW = mybir.dt.float16
    ALU = mybir.AluOpType

    consts = ctx.enter_context(tc.tile_pool(name="consts", bufs=1))
    states = ctx.enter_context(tc.tile_pool(name="states", bufs=6))
    temps = ctx.enter_context(tc.tile_pool(name="temps", bufs=6))
    psums = ctx.enter_context(tc.tile_pool(name="psums", bufs=4, space="PSUM"))
    pvs = ctx.enter_context(tc.tile_pool(name="pvs", bufs=2, space="PSUM"))
    tpsums = ctx.enter_context(tc.tile_pool(name="tpsums", bufs=1, space="PSUM"))

    offs1, packw1 = pack_offsets(PACK1_ORDER)
    offs2, packw2 = pack_offsets(PACK2_ORDER)
    CP = consts.tile([128, packw1], FW, name="CP", uniquify=False)
    UP = consts.tile([128, packw2], FW, name="UP", uniquify=False)

    def cslice(nm):
        if nm in offs1:
            o, w = offs1[nm]
            return CP[:, o:o + w]
        o, w = offs2[nm]
        return UP[:, o:o + w]

    # diag tables
    t1b = consts.tile([128, 2048], FW, name="c_t1b", uniquify=False)
    t2b = consts.tile([128, 2048], FW, name="c_t2b", uniquify=False)
    t1a = consts.tile([128, 2048], FW, name="c_t1a", uniquify=False)
    t2a = consts.tile([128, 2048], FW, name="c_t2a", uniquify=False)
    # load order: t1b (layer-0 shortcut) first, then packs, then the rest
    for q in range(4):
        qs = slice(512 * q, 512 * q + 512)
        nc.sync.dma_start(t1b[:, qs], dram["t1b"][:, qs])
    nc.sync.dma_start(CP[:, :], dram["cpack"][:, :])
    nc.sync.dma_start(UP[:, :], dram["upack"][:, :])
    nc.sync.dma_start(t1a[:, :], dram["t1a"][:, :])
    nc.sync.dma_start(t2a[:, :], dram["t2a"][:, :])
    nc.sync.dma_start(t2b[:, :], dram["t2b"][:, :])

    k2l = cslice("k2l")
    k2r = [cslice(f"k2r_{k}") for k in range(3)]
    of = [cslice(f"of_{k}") for k in range(4)]

    def cpair(nm):
        # 256-wide slice covering two adjacent pack entries
        o, _ = offs2[nm]
        return UP[:, o:o + 256]

    # ufp[l][0] = [uf0|uf2] (T1 tables, both branches), ufp[l][1] = [uf1|uf3]
    ufp = [[cpair(f"uf_{l}_0"), cpair(f"uf_{l}_1")] for l in range(3)]
    v0 = [[cslice(f"v0_{l}_{k}") for k in range(3)] for l in range(3)]
    v1 = [[cslice(f"v1_{l}_{k}") for k in range(3)] for l in range(3)]

    ones = consts.tile([128, 1], FW, name="ones", uniquify=False)
    nc.vector.memset(ones[:, :], 1.0)

    NS = 4
    FQ = 512

    def t1b_h(tbl, h):
        # per-stream contiguous [128,512] block, (r,c4,i)-major inside
        return tbl[:, 512 * h:512 * h + 512]

    def rci(t2d):
        return t2d[:, :].rearrange("p (r c i) -> p r c i", r=2, i=64)

    def cri(t2d):
        return t2d[:, :].rearrange("p (c r i) -> p r c i", r=2, i=64)

    def jmult(trip, src, dst_psum):
        # src (r,c,i)-major oldB state [128,512]; Wr serves both planes in
        # one paired N=512 matmul (out re|im adjacent, rhs re|im adjacent)
        WrT, WiT, WmiT = trip
        sre, sim = src[:, 0:256], src[:, 256:512]
        o_re = dst_psum[:, 0:256]
        o_im = dst_psum[:, 256:512]
        nc.tensor.matmul(dst_psum[:, 0:512], WrT, src[:, 0:512],
                         start=True, stop=False, skip_group_check=True)
        nc.tensor.matmul(o_re, WmiT, sim, start=False, stop=True,
                         skip_group_check=True)
        nc.tensor.matmul(o_im, WiT, sre, start=False, stop=True,
                         skip_group_check=True)

    def bplanes(t):
        # oldB (c,r,i)-major tile -> plane views [128, 4, 64] strided
        v = t[:, :].rearrange("p (c r i) -> p r c i", r=2, i=64)
        return v[:, 0], v[:, 1]

    def jmult2(trip0, src0, trip1, src1, dst_psum):
        # Wr of each branch serves both output planes via an r-major-strided
        # full-width rhs view; only the Wi cross terms stay per-plane
        s0re, s0im = bplanes(src0)
        s1re, s1im = bplanes(src1)
        s0rm = src0[:, :].rearrange("p (c r i) -> p r c i", r=2, i=64)
        s1rm = src1[:, :].rearrange("p (c r i) -> p r c i", r=2, i=64)
        o_re = dst_psum[:, 0:256]
        o_im = dst_psum[:, 256:512]
        full = dst_psum[:, 0:512]
        nc.tensor.matmul(full, trip0[0], s0rm, start=True, stop=False,
                         skip_group_check=True)
        nc.tensor.matmul(full, trip1[0], s1rm, start=False, stop=False,
                         skip_group_check=True)
        nc.tensor.matmul(o_re, trip0[2], s0im, start=False, stop=False,
                         skip_group_check=True)
        nc.tensor.matmul(o_re, trip1[2], s1im, start=False, stop=True,
                         skip_group_check=True)
        nc.tensor.matmul(o_im, trip0[1], s0re, start=False, stop=False,
                         skip_group_check=True)
        nc.tensor.matmul(o_im, trip1[1], s1re, start=False, stop=True,
                         skip_group_check=True)

    ident = cslice("ident")

    def flip(src, dst, copy_eng):
        # 4 back-to-back PE transposes (ident stays stationary), then one
        # PSUM->SBUF copy on the given engine
        tp = tpsums.tile([128, FQ], FW, name="tp", tag="tp")
        for c in range(4):
            nc.tensor.transpose(tp[:, 128 * c:128 * c + 128],
                                src[:, 128 * c:128 * c + 128], ident)
        copy_eng(dst[:, :], tp[:, :])

    # ---------------- pipeline ------------------------------------------
    pv_tiles = [None] * NS
    res = states.tile([1, 32], FP, name="res", tag="res", bufs=1)

    for l in range(4):
        SB2s = [None] * NS
        SB3s = [None] * NS
        SAs = [None] * NS
        T1s = [None] * NS
        T2s = [None] * NS
        for h in range(NS):
            SB2s[h] = states.tile([128, FQ], FW, name=f"SB2_{h}", tag="SB2")
            if l == 0:
                nc.scalar.mul(SB2s[h][:, :], t1b_h(t1b, h), 1.0 / 64.0)
            else:
                pv = pv_tiles[h]
                d1 = temps.tile([128, FQ], FW, name="d1", tag="d1")
                d2 = temps.tile([128, FQ], FW, name="d2", tag="d2")
                nc.vector.tensor_tensor(d1[:, :], pv[:, :], t1b_h(t1b, h),
                                        ALU.mult)
                nc.vector.tensor_tensor(d2[:, :], pv[:, :], t1b_h(t2b, h),
                                        ALU.mult)
                nc.gpsimd.tensor_tensor(SB2s[h][:, 0:256], d1[:, 0:256],
                                        d1[:, 256:512], ALU.subtract)
                nc.gpsimd.tensor_tensor(SB2s[h][:, 256:512], d2[:, 0:256],
                                        d2[:, 256:512], ALU.add)
        for h in range(NS):
            pk = psums.tile([128, FQ], FP, name=f"pk{h}", tag="pstate")
            jmult(k2r, SB2s[h], pk)
            SB3s[h] = states.tile([128, FQ], FW, name=f"SB3_{h}", tag="SB3")
            nc.scalar.copy(cri(SB3s[h]), rci(pk))
        for h in range(NS):
            SAs[h] = states.tile([128, FQ], FW, name=f"SA{h}", tag="SA")
            flip(SB3s[h], SAs[h], nc.scalar.copy)
        for h in range(NS):
            pl = psums.tile([128, FQ], FP, name=f"pl{h}", tag="pstate")
            nc.tensor.matmul(pl[:, :], k2l, SAs[h][:, :], start=True, stop=True)
            hs = slice(FQ * h, FQ * h + FQ)
            T1s[h] = temps.tile([128, FQ], FW, name="t1s", tag="t1s")
            T2s[h] = temps.tile([128, FQ], FW, name="t2s", tag="t2s")
            nc.vector.tensor_tensor(T1s[h][:, :], pl[:, :], t1a[:, hs], ALU.mult)
            nc.vector.tensor_tensor(T2s[h][:, :], pl[:, :], t2a[:, hs], ALU.mult)

        if l == 3:
            for h in range(NS):
                po1 = psums.tile([128, FQ], FP, name=f"po1{h}", tag="pstate")
                nc.tensor.matmul(po1[:, :], of[0], T1s[h][:, :], start=True,
                                 stop=False)
                nc.tensor.matmul(po1[:, :], of[1], T2s[h][:, :], start=False,
                                 stop=True)
                po2 = pvs.tile([128, FQ], FP, name=f"po2{h}", tag="pv")
                nc.tensor.matmul(po2[:, :], of[2], T1s[h][:, :], start=True,
                                 stop=False)
                nc.tensor.matmul(po2[:, :], of[3], T2s[h][:, :], start=False,
                                 stop=True)
                PR1 = states.tile([128, FQ], FW, name=f"PR1{h}", tag="SB3")
                PR2 = states.tile([128, FQ], FW, name=f"PR2{h}", tag="SA")
                nc.vector.tensor_tensor(PR1[:, :], T1s[h][:, :], po1[:, :],
                                        ALU.mult)
                nc.vector.tensor_tensor(PR2[:, :], T2s[h][:, :], po2[:, :],
                                        ALU.mult)
                ep = tpsums.tile([1, FQ], FP, name="ep", tag="ep", bufs=1)
                nc.tensor.matmul(ep[:, :], ones[:, :], PR1[:, :], start=True,
                                 stop=False)
                nc.tensor.matmul(ep[:, :], ones[:, :], PR2[:, :], start=False,
                                 stop=True)
                epv = ep[:, :].rearrange("p (b j) -> p b j", j=64)
                nc.vector.tensor_reduce(res[:, 8 * h:8 * h + 8], epv,
                                        axis=mybir.AxisListType.X, op=ALU.add)
        else:
            SBas = [None] * NS
            SBbs = [None] * NS
            for h in range(NS):
                # fused U-side: stationary = D2-product chunk, rhs = paired
                # folded tables; out chunk = [flip(U0-res) | flip(U1-res)]
                pbx1 = psums.tile([128, FQ], FP, name=f"pbx1{h}", tag="pstate")
                pbx2 = psums.tile([128, FQ], FP, name=f"pbx2{h}", tag="pstate")
                for c in range(4):
                    cs = slice(128 * c, 128 * c + 128)
                    pbx = pbx1 if c < 2 else pbx2
                    os_ = slice(256 * (c % 2), 256 * (c % 2) + 256)
                    nc.tensor.matmul(pbx[:, os_], T1s[h][:, cs], ufp[l][0],
                                     start=True, stop=False)
                    nc.tensor.matmul(pbx[:, os_], T2s[h][:, cs], ufp[l][1],
                                     start=False, stop=True)
                SBas[h] = states.tile([128, FQ], FW, name=f"SBa{h}", tag="SB3")
                SBbs[h] = states.tile([128, FQ], FW, name=f"SBb{h}", tag="SA")
                v1_ = pbx1[:, :].rearrange("p (cc s i) -> p s cc i", s=2, i=128)
                v2_ = pbx2[:, :].rearrange("p (cc s i) -> p s cc i", s=2, i=128)
                a_ = SBas[h][:, :].rearrange("p (cc i) -> p cc i", i=128)
                b_ = SBbs[h][:, :].rearrange("p (cc i) -> p cc i", i=128)
                nc.scalar.copy(a_[:, 0:2], v1_[:, 0])
                nc.scalar.copy(a_[:, 2:4], v2_[:, 0])
                nc.scalar.copy(b_[:, 0:2], v1_[:, 1])
                nc.scalar.copy(b_[:, 2:4], v2_[:, 1])
            for h in range(NS):
                pv = pvs.tile([128, FQ], FP, name=f"pv{h}", tag="pv")
                jmult2(v0[l], SBas[h], v1[l], SBbs[h], pv)
                pv_tiles[h] = pv

    nc.sync.dma_start(dram["out"][:, :], res[:, :])


# ======================================================================
# public entry point
# ======================================================================
_CACHED = {}


def _build_program(use_f32r=True):
    """Build + compile the (input-independent) bass program once."""
    key = True
    if key in _CACHED:
        return _CACHED[key]
    from contextlib import ExitStack
    import concourse.bacc as bacc
    import concourse.mybir as mybir
    import concourse.tile as tile

    nc = bacc.Bacc("TRN2", target_bir_lowering=False, debug=False,
                   enable_asserts=True)
    _, packw1 = pack_offsets(PACK1_ORDER)
    _, packw2 = pack_offsets(PACK2_ORDER)
    shapes = {"cpack": [128, packw1], "upack": [128, packw2],
              "t1b": [128, 2048], "t2b": [128, 2048],
              "t1a": [128, 2048], "t2a": [128, 2048]}
    dram = {}
    for name, shape in shapes.items():
        dram[name] = nc.dram_tensor(
            name, shape, mybir.dt.float16, kind="ExternalInput").ap()
    dram["out"] = nc.dram_tensor("out", [1, 32], mybir.dt.float32,
                                 kind="ExternalOutput").ap()
    with tile.TileContext(nc) as tc:
        with ExitStack() as ctx:
            emit(ctx, tc, dram)
    nc.compile()
    _CACHED[key] = nc
    return nc


def kernel(x, params):
    """Full-input entry point: x (256,) f32, params (144,) f32 -> (256,) f32.

    Shards the batch over 8 NeuronCores (32 per core), runs the Bass/Tile
    statevector kernel SPMD, gathers per-core expectation values.
    """
    from concourse.bass_utils import run_bass_kernel_spmd

    x = np.asarray(x, dtype=np.float32).reshape(BATCH)
    params = np.asarray(params, dtype=np.float32).reshape(N_QUBITS * N_LAYERS * 3)
    nc = _build_program()
    in_maps = build_host_data(x, params)
    res = run_bass_kernel_spmd(nc, in_maps, list(range(N_CORES)))
    out = np.concatenate([res.results[c]["out"].reshape(BPC)
                          for c in range(N_CORES)])
    return out.astype(np.float32)
